# revision 41
# baseline (speedup 1.0000x reference)
"""AncProbsLayer Trainium2 kernel (8 NeuronCores, data-parallel over batch).

out[b,l,h,0,z] = sum_d seq[b,l,0,0,d] * P[b,h,d,z]
P[b,h] = diag(1/sqrt_pi_h) V_h diag(exp(lam_h * tau[b,h])) V_h^T diag(sqrt_pi_h)

The GTR eigendecomposition (H=8 independent symmetric 20x20 eigh) is
precomputed on host (the reference marks it "constant (non-trainable)");
the per-(b,h) transition matrices P are assembled on host (tiny: 1.6 MB)
and the device does the memory-bound part: the (B*L, 20) x (20, 160)
batched matmul producing the 160 MiB output.

Device structure (per core: BS=16 batches, paired):
- batches processed in PAIRS: lhsT stacks both batches' 20 d-rows (K=40),
  rhs is the block-diagonal [[W_b0, 0], [0, W_b1]] (40, 320), so one
  LDWEIGHTS+MATMUL computes a 128-row l-tile for BOTH batches (N=320
  fits one PSUM bank). Halves the LDWEIGHTS-paced PE time.
- l rows are interleaved on host so PSUM partition p, tile t holds row
  16p+t: each SBUF partition's 16 tiles form one contiguous 10KB run in
  the (paired-layout) output -> line-rate DMA descriptors.
- PSUM -> SBUF copies (mandatory: DMA can't read PSUM) are split between
  the Vector and Scalar engines, casting f32 -> bf16; the bf16 output is
  upcast to f32 on host (rel-err ~2e-3, well under the 2e-2 gate).
"""

import sys

sys.path.insert(0, "/opt/trn_rl_repo")
sys.path.insert(0, "/root/.axon_site")

import numpy as np


def _install_axon_hooks_shim():
    """The agent image's antenv lacks axon_hooks; bass_utils imports it when
    BASS_TRACE=1. Provide it (registering the ctypes NTFF hook if possible)
    so tracing degrades gracefully instead of crashing."""
    try:
        import antenv.axon_hooks  # noqa: F401

        return
    except ImportError:
        pass
    try:
        import types

        mod = types.ModuleType("antenv.axon_hooks")
        _h = [None]
        mod.set_axon_ntff_profile_hook = lambda h: _h.__setitem__(0, h)
        mod.get_axon_ntff_profile_hook = lambda: _h[0]
        sys.modules["antenv.axon_hooks"] = mod
        import antenv

        antenv.axon_hooks = mod
        try:
            from trn_agent_boot.trn_boot import _ntff_profile_via_ctypes

            mod.set_axon_ntff_profile_hook(
                _ntff_profile_via_ctypes("/opt/axon/libaxon_pjrt.so")
            )
        except Exception:
            pass
    except Exception:
        pass


_install_axon_hooks_shim()

B, L, H, D = 128, 2048, 8, 20
N_CORES = 8
BS = B // N_CORES  # batches per core
PAIRS = BS // 2
HZ = H * D  # 160 output cols per (b, l)
LT = L // 128  # l-tiles per batch

_NC = None  # compiled Bass graph cache
LAST_RESULTS = None  # BassKernelResults of the most recent run (for profiling)
LAST_IN_MAPS = None  # per-core input shards of the most recent run


def _build_nc():
    import concourse.bacc as bacc
    import concourse.tile as tile
    import concourse.mybir as mybir

    f32 = mybir.dt.float32
    bf16 = mybir.dt.bfloat16
    nc = bacc.Bacc(None, target_bir_lowering=False)

    seqt = nc.declare_dram_parameter("seqt", [2 * D, PAIRS * L], bf16, isOutput=False)
    w = nc.declare_dram_parameter("w", [2 * D, PAIRS * 2 * HZ], bf16, isOutput=False)
    out = nc.declare_dram_parameter("out", [PAIRS * L, 2 * HZ], bf16, isOutput=True)

    with tile.TileContext(nc) as tc:
        with (
            tc.tile_pool(name="wpool", bufs=1) as wpool,
            tc.tile_pool(name="spool", bufs=10) as spool,
            tc.tile_pool(name="psum", bufs=8, space="PSUM") as ppool,
            tc.tile_pool(name="opool", bufs=5) as opool,
        ):
            wt = wpool.tile([2 * D, PAIRS * 2 * HZ], bf16)
            for pr in range(PAIRS):
                nc.sync.dma_start(
                    wt[:, pr * 2 * HZ : (pr + 1) * 2 * HZ],
                    w[:, pr * 2 * HZ : (pr + 1) * 2 * HZ],
                )
            for pr in range(PAIRS):
                # two half-tiles per pair so the first matmuls only wait on
                # an 80KB load instead of the full 160KB
                sh = []
                for j in range(2):
                    stj = spool.tile([2 * D, L // 2], bf16, tag="st")
                    nc.gpsimd.dma_start(
                        stj[:], seqt[:, pr * L + j * (L // 2) : pr * L + (j + 1) * (L // 2)]
                    )
                    sh.append(stj)
                ot = opool.tile([128, LT, 2 * HZ], bf16)
                for t in range(LT):
                    st = sh[t // (LT // 2)]
                    tt = t % (LT // 2)
                    ps = ppool.tile([128, 2 * HZ], f32)
                    nc.tensor.matmul(
                        ps[:],
                        st[:, tt * 128 : (tt + 1) * 128],
                        wt[:, pr * 2 * HZ : (pr + 1) * 2 * HZ],
                        start=True,
                        stop=True,
                    )
                    # PSUM->SBUF cast copies split across both capable engines
                    if (pr * LT + t) % 2 == 0:
                        nc.vector.tensor_copy(ot[:, t, :], ps[:])
                    else:
                        nc.scalar.copy(ot[:, t, :], ps[:])
                # partition p, tile t holds original row l=16p+t of both
                # batches: contiguous 16*320*2B = 10KB per partition.
                dst = out[pr * L : (pr + 1) * L, :].rearrange(
                    "(p t) zz -> p t zz", p=128, t=LT
                )
                # first pair: quarter-split so the output stream starts as
                # early as possible; last pairs: quarter-split so the tail
                # drains with more queue parallelism
                nsplit = 4 if (pr == 0 or pr >= PAIRS - 2) else 2
                chunk = LT // nsplit
                for j in range(nsplit):
                    nc.sync.dma_start(
                        dst[:, j * chunk : (j + 1) * chunk, :],
                        ot[:, j * chunk : (j + 1) * chunk, :],
                    )
    nc.compile()
    return nc


def _get_nc():
    global _NC
    if _NC is None:
        _NC = _build_nc()
    return _NC


def _host_precompute(rate_indices, tau_kernel, exchangeability_kernel, equilibrium_kernel):
    """Everything up to the per-(b,h) 20x20 transition matrices, in float64."""
    ek = exchangeability_kernel.astype(np.float64)[:, 0]  # (H, D, D)
    eq = equilibrium_kernel.astype(np.float64)[:, 0]  # (H, D)

    R = np.logaddexp(ek, 0.0)
    R = 0.5 * (R + R.transpose(0, 2, 1))
    m = eq.max(axis=-1, keepdims=True)
    p = np.exp(eq - m)
    p /= p.sum(axis=-1, keepdims=True)

    Q = R * p[:, None, :]
    diag = Q.sum(axis=-1)
    Q = Q - diag[:, :, None] * np.eye(D)
    mue = (p * diag).sum(axis=-1)
    Q = Q / np.maximum(mue, 1e-16)[:, None, None]

    sq = np.sqrt(p)
    isq = 1.0 / sq
    S = sq[:, :, None] * Q * isq[:, None, :]
    S = 0.5 * (S + S.transpose(0, 2, 1))
    lam, V = np.linalg.eigh(S)  # (H, D), (H, D, D)

    W1 = isq[:, :, None] * V  # (H, D, D): rows d, cols k
    W2 = V.transpose(0, 2, 1) * sq[:, None, :]  # (H, D, D): rows k, cols z

    tau_g = tau_kernel[rate_indices, np.arange(H)[None, :], 0].astype(np.float64)
    tau = np.logaddexp(np.clip(tau_g, -80.0, 80.0), 0.0)  # (B, H)
    e = np.exp(lam[None, :, :] * tau[:, :, None])  # (B, H, D)

    P = np.einsum("hdk,bhk,hkz->bhdz", W1, e, W2)  # (B, H, D, D)
    # w[b][d, h*20+z] = P[b,h,d,z]
    return np.ascontiguousarray(P.transpose(0, 2, 1, 3)).reshape(B, D, HZ).astype(np.float32)


def kernel(sequences, rate_indices, tau_kernel, exchangeability_kernel, equilibrium_kernel):
    global LAST_RESULTS, LAST_IN_MAPS
    from concourse.bass_utils import run_bass_kernel_spmd
    import ml_dtypes

    sequences = np.asarray(sequences)
    rate_indices = np.asarray(rate_indices)
    tau_kernel = np.asarray(tau_kernel)
    exchangeability_kernel = np.asarray(exchangeability_kernel)
    equilibrium_kernel = np.asarray(equilibrium_kernel)

    w_all = _host_precompute(
        rate_indices, tau_kernel, exchangeability_kernel, equilibrium_kernel
    )
    seq = np.asarray(sequences, dtype=np.float32).reshape(B, L, D)

    # interleave l within each batch: device l-tile t, position q <- row 16q+t
    # (so each psum partition's 16 tiles land contiguous in the output)
    seq_il = seq.reshape(B, L // 16, 16, D).transpose(0, 2, 1, 3).reshape(B, L, D)

    in_maps = []
    for c in range(N_CORES):
        b0 = c * BS
        # seqt: (40, PAIRS*L); rows 0-19 = even batch of the pair, 20-39 = odd
        seqt = np.zeros((2 * D, PAIRS * L), dtype=ml_dtypes.bfloat16)
        wc = np.zeros((2 * D, PAIRS * 2 * HZ), dtype=ml_dtypes.bfloat16)
        for pr in range(PAIRS):
            for k in range(2):
                b = b0 + 2 * pr + k
                seqt[k * D : (k + 1) * D, pr * L : (pr + 1) * L] = seq_il[b].T
                wc[
                    k * D : (k + 1) * D,
                    pr * 2 * HZ + k * HZ : pr * 2 * HZ + (k + 1) * HZ,
                ] = w_all[b]
        in_maps.append({"seqt": seqt, "w": wc})

    LAST_IN_MAPS = in_maps
    nc = _get_nc()
    res = run_bass_kernel_spmd(nc, in_maps, core_ids=list(range(N_CORES)))
    LAST_RESULTS = res

    # device out: (PAIRS*L, 2, HZ) bf16 -> (BS, L, H, D) f32 per core
    outs = []
    for c in range(N_CORES):
        o = res.results[c]["out"].astype(np.float32).reshape(PAIRS, L, 2, HZ)
        outs.append(o.transpose(0, 2, 1, 3).reshape(BS, L, H, D))
    out = np.concatenate(outs, axis=0)
    return np.ascontiguousarray(out.reshape(B, L, H, 1, D))
